# revision 16
# baseline (speedup 1.0000x reference)
"""DimeNet-style GNN message passing on 8 Trainium2 NeuronCores.

Sharding: edges are packed into 128-edge "windows" such that each window's
triplet count <= K_FIX*128; windows are dealt to 8 cores (graph-parallel).
Each core owns its edges AND all triplets targeting them (gather and
scatter in the interaction block both use idx_kj, so triplet work is fully
local to the owning core).  Gather (x_kj[idx_kj]) is an expand-matmul with
a one-hot matrix; scatter-add is a matmul with the transposed one-hot,
accumulated in PSUM per window.  The only cross-core communication is one
ReduceScatter of the [H, N] atom-message partial sums.
"""
import os
import sys
import numpy as np

sys.path.insert(0, "/opt/trn_rl_repo")

H = 128
NR = 16
NS = 6
L = 2
CUTOFF = 8.0
NCORES = 8
TWO_PI = float(2 * np.pi)
F32 = np.float32
LAST_RESULTS = None


# ----------------------------------------------------------------------------
# host-side helpers
# ----------------------------------------------------------------------------

def _envelope(x):
    x5 = x ** 5
    return np.where(x < 1.0, 1.0 / x - 28.0 * x5 + 48.0 * x5 * x - 21.0 * x5 * x * x, 0.0)


def _pack_edges(deg, n_windows):
    """Deal edges (sorted by degree desc) snake-wise into n_windows windows.
    Returns list of edge-id lists. Balances both edge count and triplet sum."""
    order = np.argsort(-deg, kind="stable")
    wins = [[] for _ in range(n_windows)]
    i = 0
    fwd = True
    for e in order:
        w = i if fwd else n_windows - 1 - i
        wins[w].append(int(e))
        i += 1
        if i == n_windows:
            i = 0
            fwd = not fwd
    return wins


def kernel(**inputs):
    import concourse.bass as bass
    import concourse.bacc as bacc
    import concourse.mybir as mybir
    import concourse.tile as tile
    from concourse.bass import IndirectOffsetOnAxis
    from concourse.bass_utils import run_bass_kernel_spmd

    DT = mybir.dt.float32

    af = np.asarray(inputs["atom_feature"], F32)     # [N,133]
    ef = np.asarray(inputs["edge_feature"], F32)     # [E,14]
    dist = np.asarray(inputs["dist"], F32)           # [E]
    angle = np.asarray(inputs["angle"], F32)         # [T]
    i_idx = np.asarray(inputs["i"]).astype(np.int64)
    j_idx = np.asarray(inputs["j"]).astype(np.int64)
    idx_kj = np.asarray(inputs["idx_kj"]).astype(np.int64)
    ib_eid = np.asarray(inputs["incomebond_edge_ids"]).astype(np.int64)
    ib_atom = np.asarray(inputs["incomebond_index_to_atom"]).astype(np.int64)

    N, FA = af.shape
    E = ef.shape[0]
    T = angle.shape[0]
    FE = ef.shape[1]
    FI = FA + FE                                     # 147

    # --- host precompute (index decode / input gathers / tiny per-edge scalars)
    atom_type = np.argmax(af[:, :100], axis=1)
    x_emb = np.asarray(inputs["emb_table"], F32)[atom_type]          # [N,H]
    d_edge = (dist / CUTOFF).astype(F32)                             # [E]
    env_edge = _envelope(d_edge.astype(np.float64)).astype(F32)      # [E]

    # --- edge -> window packing
    deg = np.bincount(idx_kj, minlength=E)
    # windows total: multiple of 32 (so NW per core is a multiple of 4) and
    # enough that the average triplets/window leaves headroom under 512
    NW_TOT = -(-(-(-E // 128)) // 32) * 32
    while T / NW_TOT > 490.0:
        NW_TOT += 32
    wins = _pack_edges(deg, NW_TOT)
    tmax = max(int(deg[w].sum()) for w in wins if w)
    K_FIX = max(1, -(-tmax // 128))
    TPW = 128 * K_FIX                                # triplet slots per window
    NW = NW_TOT // NCORES                            # windows per core
    EC = NW * 128                                    # edge slots per core
    NSC = -(-NW // 4)                                # superchunks of 4 windows
    assert NSC * 4 == NW, (NW,)

    # deal windows to cores (snake by triplet load)
    wloads = np.array([int(deg[w].sum()) for w in wins])
    worder = np.argsort(-wloads, kind="stable")
    core_wins = [[] for _ in range(NCORES)]
    i = 0
    fwd = True
    for w in worder:
        c = i if fwd else NCORES - 1 - i
        core_wins[c].append(int(w))
        i += 1
        if i == NCORES:
            i = 0
            fwd = not fwd

    # triplets grouped by target edge
    t_order = np.argsort(idx_kj, kind="stable")
    t_sorted_edge = idx_kj[t_order]
    seg_starts = np.searchsorted(t_sorted_edge, np.arange(E))
    seg_ends = np.searchsorted(t_sorted_edge, np.arange(E), side="right")

    owner = np.full(E, -1, np.int32)
    localrow = np.full(E, -1, np.int32)

    per_core = []
    for c in range(NCORES):
        edge_ids = np.full(EC, -1, np.int64)
        for wl, w in enumerate(core_wins[c]):
            es = wins[w]
            edge_ids[wl * 128: wl * 128 + len(es)] = es
        real = edge_ids >= 0
        re = edge_ids[real]
        owner[re] = c
        localrow[re] = np.nonzero(real)[0].astype(np.int32)

        # per-edge device inputs (feature-major, padded edges -> 0)
        ibT = np.zeros((FI, EC), F32)
        embiT = np.zeros((H, EC), F32)
        embjT = np.zeros((H, EC), F32)
        dE = np.full((1, EC), 0.5, F32)
        envE = np.zeros((1, EC), F32)
        ibT[:FA, real] = af[j_idx[re]].T
        ibT[FA:, real] = ef[re].T
        embiT[:, real] = x_emb[i_idx[re]].T
        embjT[:, real] = x_emb[j_idx[re]].T
        dE[0, real] = d_edge[re]
        envE[0, real] = env_edge[re]

        # triplet slots
        TP = NW * TPW
        tripmeta = np.zeros((4, TP), F32)            # angle, dkj, envkj, segrel
        tripmeta[1] = 0.5
        tripmeta[3] = -1.0
        for wl in range(NW):
            pos = wl * TPW
            for p in range(128):
                e = edge_ids[wl * 128 + p]
                if e < 0:
                    continue
                ts = t_order[seg_starts[e]:seg_ends[e]]
                n = len(ts)
                if n == 0:
                    continue
                tripmeta[0, pos:pos + n] = angle[ts]
                tripmeta[1, pos:pos + n] = d_edge[e]
                tripmeta[2, pos:pos + n] = env_edge[e]
                tripmeta[3, pos:pos + n] = float(p)
                pos += n
            assert pos <= (wl + 1) * TPW
        # segrel transposed into columns of 128 for the scatter one-hot
        segcolT = np.ascontiguousarray(
            tripmeta[3].reshape(NW * K_FIX, 128).T)   # [128, NW*K_FIX]
        # one row per window: [angle | dkj | envkj | segrel] concatenated
        tripcat = np.ascontiguousarray(
            tripmeta.reshape(4, NW, TPW).transpose(1, 0, 2).reshape(1, NW * 4 * TPW))
        per_core.append(dict(ibT=ibT, embiT=embiT, embjT=embjT, dE=dE, envE=envE,
                             tripcat=tripcat, segcolT=segcolT,
                             edge_ids=edge_ids))

    # --- income bonds -> owner of source edge, laid out by target-atom window
    # atom windows: multiple of 32 so each core's final shard is a multiple of 512
    NAW = -(-(-(-N // 128)) // 32) * 32
    NA = NAW * 128                                   # padded atom count
    ASH = NA // NCORES                               # atoms per core for final
    bond_owner = owner[ib_eid]
    counts = np.zeros((NCORES, NAW), np.int64)
    for c in range(NCORES):
        sel = np.nonzero(bond_owner == c)[0]
        w_of = ib_atom[sel] // 128
        cnt = np.bincount(w_of, minlength=NAW)
        counts[c] = cnt
    K_A = max(1, -(-int(counts.max()) // 128))
    BPW = 128 * K_A
    BP = NAW * BPW
    for c in range(NCORES):
        srwar = np.zeros((1, BP), np.int32)
        tgw = np.full((1, BP), -1.0, F32)
        sel = np.nonzero(bond_owner == c)[0]
        aw = ib_atom[sel] // 128
        order2 = np.argsort(aw, kind="stable")
        sel = sel[order2]
        aw = aw[order2]
        starts = np.searchsorted(aw, np.arange(NAW))
        ends = np.searchsorted(aw, np.arange(NAW), side="right")
        for w in range(NAW):
            b = sel[starts[w]:ends[w]]
            n = len(b)
            srwar[0, w * BPW: w * BPW + n] = localrow[ib_eid[b]]
            tgw[0, w * BPW: w * BPW + n] = (ib_atom[b] - 128 * w).astype(F32)
        per_core[c]["srcrow"] = np.ascontiguousarray(srwar.reshape(NAW * K_A, 128).T)
        per_core[c]["tgtrel"] = np.ascontiguousarray(tgw.reshape(NAW * K_A, 128).T)
        afT = np.zeros((FA, ASH), F32)
        lo = c * ASH
        hi = min(N, lo + ASH)
        if hi > lo:
            afT[:, :hi - lo] = af[lo:hi].T
        per_core[c]["afT"] = afT

    # --- replicated weights / constants
    W = {k: np.asarray(v, F32) for k, v in inputs.items()
         if k not in ("atom_feature", "edge_feature", "dist", "angle", "i", "j",
                      "idx_kj", "idx_ji", "incomebond_edge_ids",
                      "incomebond_index_to_atom")}
    bf = W["bessel_freq"]                            # [NR] = pi*(1..NR)
    const = dict(
        ones512=np.ones((1, 512), F32),
        q025=np.full((1, NS), 0.25, F32),
        svecn=(np.arange(NS, dtype=F32) / TWO_PI).reshape(1, NS),
        freqn=(bf / TWO_PI).reshape(1, NR).astype(F32),
        iota_mat=np.tile(np.arange(128, dtype=F32), (128, 1)),
        iota_col=np.arange(128, dtype=F32).reshape(128, 1),
        identity=np.eye(128, dtype=F32),
        Wi1a=W["W_i1_w"][:128], Wi1b=W["W_i1_w"][128:FI],
        b_i1=W["W_i1_b"].reshape(H, 1),
        Wrbf=W["lin_rbf_w"], b_rbf=W["lin_rbf_b"].reshape(H, 1),
        Wemb_i=W["lin_emb_w"][:H], Wemb_j=W["lin_emb_w"][H:2 * H],
        Wemb_r=W["lin_emb_w"][2 * H:], b_emb=W["lin_emb_b"].reshape(H, 1),
        Woa1=W["W_o_w"][:128], Woa2=W["W_o_w"][128:FA],
        Wom=W["W_o_w"][FA:], b_o=W["W_o_b"].reshape(H, 1),
        # REP6[r, s*16+r'] = delta(r,r');  REPC[s', s*16+r] = delta(s,s')
        REP6=np.tile(np.eye(NR, dtype=F32), (1, NS)),
        REPC=np.repeat(np.eye(NS, dtype=F32), NR, axis=1),
    )
    for l in range(L):
        const[f"Wkj{l}"] = W["L_kj_w"][l]
        const[f"b_kj{l}"] = W["L_kj_b"][l].reshape(H, 1)
        const[f"Wrbf2{l}"] = W["L_rbf2_w"][l]
        const[f"b_rbf2r{l}"] = W["L_rbf2_b"][l].reshape(1, H)
        const[f"Wsbf1{l}"] = W["L_sbf1_w"][l]
        const[f"Wsbf2{l}"] = W["L_sbf2_w"][l]
        const[f"Wdown{l}"] = W["L_down_w"][l]
        const[f"bdownr{l}"] = W["L_down_b"][l].reshape(1, H)
        const[f"Wup{l}"] = W["L_up_w"][l]
        const[f"bupr{l}"] = W["L_up_b"][l].reshape(1, H)
        const[f"Wres1_{l}"] = W["L_res1_w"][l]
        const[f"b_res1_{l}"] = W["L_res1_b"][l].reshape(H, 1)
        const[f"Wres2_{l}"] = W["L_res2_w"][l]
        const[f"b_res2_{l}"] = W["L_res2_b"][l].reshape(H, 1)

    # ------------------------------------------------------------------
    # build the Bass program (identical for all cores)
    # ------------------------------------------------------------------
    nc = bacc.Bacc("TRN2", target_bir_lowering=False, debug=False,
                   num_devices=NCORES)

    def din(name, arr):
        return nc.dram_tensor(name, list(arr.shape), DT if arr.dtype == F32
                              else mybir.dt.int32, kind="ExternalInput")

    d_const = {k: din(k, v) for k, v in const.items()}
    p0 = per_core[0]
    d_ibT = din("ibT", p0["ibT"])
    d_embiT = din("embiT", p0["embiT"])
    d_embjT = din("embjT", p0["embjT"])
    d_dE = din("dE", p0["dE"])
    d_envE = din("envE", p0["envE"])
    d_tripcat = din("tripcat", p0["tripcat"])
    d_segcolT = din("segcolT", p0["segcolT"])
    d_srcrow = din("srcrow", p0["srcrow"])
    d_tgtrel = din("tgtrel", p0["tgtrel"])
    d_afT = din("afT", p0["afT"])
    d_out = nc.dram_tensor("outT", [H, ASH], DT, kind="ExternalOutput")

    TP = NW * TPW

    with tile.TileContext(nc) as tc:
        with (
            tc.tile_pool(name="const", bufs=1) as cpool,
            tc.tile_pool(name="sb", bufs=4) as sb,
            tc.tile_pool(name="sbsmall", bufs=4) as sbs,
            tc.tile_pool(name="psb", bufs=2, space="PSUM") as psb,      # [128,512]
            tc.tile_pool(name="pss", bufs=4, space="PSUM") as pss,      # [128,128]
            tc.tile_pool(name="psagg", bufs=1, space="PSUM") as psagg,  # agg
            tc.tile_pool(name="dram", bufs=1, space="DRAM") as dram,
        ):
            C = {}
            for k, v in const.items():
                t = cpool.tile(list(v.shape), DT, tag=k)
                nc.sync.dma_start(t[:], d_const[k][:])
                C[k] = t

            msg = [dram.tile([H, EC], DT, tag="msgA", name="msgA"),
                   dram.tile([H, EC], DT, tag="msgB", name="msgB")]
            rbfeT = dram.tile([H, EC], DT, tag="rbfeT")
            msgRM = dram.tile([EC, H], DT, tag="msgRM")
            apart = dram.tile([NCORES, H, ASH], DT, tag="apart")
            asum = dram.tile([H, ASH], DT, tag="asum")

            RELU = mybir.ActivationFunctionType.Relu
            SIN = mybir.ActivationFunctionType.Sin
            ADD = mybir.AluOpType.add
            MULT = mybir.AluOpType.mult
            ISEQ = mybir.AluOpType.is_equal
            MAX = mybir.AluOpType.max

            def sin_of_psum(p_arg, parts, width, tag):
                """p_arg holds arg/(2pi); returns SBUF tile sin(arg) [parts,width]."""
                qi = sbs.tile([parts, width], mybir.dt.int32, tag=tag + "qi")
                nc.vector.tensor_copy(qi[:], p_arg[:])
                qf = sbs.tile([parts, width], DT, tag=tag + "qf")
                nc.vector.tensor_copy(qf[:], qi[:])
                y = sbs.tile([parts, width], DT, tag=tag + "y")
                nc.vector.scalar_tensor_tensor(y[:], qf[:], -1.0, p_arg[:], MULT, ADD)
                s = sbs.tile([parts, width], DT, tag=tag + "s")
                nc.scalar.activation(s[:], y[:], SIN, scale=TWO_PI)
                return s

            # ---------------- phase 0: embedding ----------------
            for sc in range(NSC):
                cs = slice(sc * 512, sc * 512 + 512)
                ib_hi = sb.tile([128, 512], DT, tag="ib_hi")
                nc.sync.dma_start(ib_hi[:], d_ibT[0:128, cs])
                ib_lo = sb.tile([FI - 128, 512], DT, tag="ib_lo")
                nc.sync.dma_start(ib_lo[:], d_ibT[128:FI, cs])
                pm = psb.tile([128, 512], DT, tag="big")
                nc.tensor.matmul(pm[:], C["Wi1a"][:], ib_hi[:], start=True, stop=False)
                nc.tensor.matmul(pm[:], C["Wi1b"][:], ib_lo[:], start=False, stop=True)
                m0 = sb.tile([128, 512], DT, tag="m0")
                nc.scalar.activation(m0[:], pm[:], RELU, bias=C["b_i1"][:])
                nc.sync.dma_start(msg[0][:, cs], m0[:])

                drow = sbs.tile([1, 512], DT, tag="drow")
                nc.sync.dma_start(drow[:], d_dE[:, cs])
                erow = sbs.tile([1, 512], DT, tag="erow")
                nc.sync.dma_start(erow[:], d_envE[:, cs])
                parg = pss.tile([NR, 512], DT, tag="small")
                nc.tensor.matmul(parg[:], C["freqn"][:], drow[:], start=True, stop=True)
                sin16 = sin_of_psum(parg, NR, 512, "e")
                penv = pss.tile([NR, 512], DT, tag="small")
                nc.tensor.matmul(penv[:], C["ones512"][:, :NR], erow[:], start=True, stop=True)
                rbf0 = sb.tile([NR, 512], DT, tag="rbf0")
                nc.vector.tensor_tensor(rbf0[:], sin16[:], penv[:], op=MULT)
                prh = psb.tile([128, 512], DT, tag="big")
                nc.tensor.matmul(prh[:], C["Wrbf"][:], rbf0[:], start=True, stop=True)
                rbfh = sb.tile([128, 512], DT, tag="rbfh")
                nc.scalar.activation(rbfh[:], prh[:], RELU, bias=C["b_rbf"][:])

                embi = sb.tile([128, 512], DT, tag="embi")
                nc.sync.dma_start(embi[:], d_embiT[:, cs])
                embj = sb.tile([128, 512], DT, tag="embj")
                nc.sync.dma_start(embj[:], d_embjT[:, cs])
                pre = psb.tile([128, 512], DT, tag="big")
                nc.tensor.matmul(pre[:], C["Wemb_i"][:], embi[:], start=True, stop=False)
                nc.tensor.matmul(pre[:], C["Wemb_j"][:], embj[:], start=False, stop=False)
                nc.tensor.matmul(pre[:], C["Wemb_r"][:], rbfh[:], start=False, stop=True)
                rbe = sb.tile([128, 512], DT, tag="rbe")
                nc.scalar.activation(rbe[:], pre[:], RELU, bias=C["b_emb"][:])
                nc.sync.dma_start(rbfeT[:, cs], rbe[:])

            # ---------------- phase 1: interaction layers ----------------
            for l in range([L, 0][os.environ.get("SKIP_P1") == "1"]):
                src, dst = msg[l % 2], msg[(l + 1) % 2]
                for sc in range(NSC):
                    cs = slice(sc * 512, sc * 512 + 512)
                    mt = sb.tile([128, 512], DT, tag="mt")
                    nc.sync.dma_start(mt[:], src[:, cs])
                    ret = sb.tile([128, 512], DT, tag="ret")
                    nc.sync.dma_start(ret[:], rbfeT[:, cs])
                    pkj = psb.tile([128, 512], DT, tag="big")
                    nc.tensor.matmul(pkj[:], C[f"Wkj{l}"][:], mt[:], start=True, stop=True)
                    kj = sb.tile([128, 512], DT, tag="kj")
                    nc.scalar.activation(kj[:], pkj[:], RELU, bias=C[f"b_kj{l}"][:])
                    pr = psb.tile([128, 512], DT, tag="big")
                    nc.tensor.matmul(pr[:], C[f"b_rbf2r{l}"][:], C["ones512"][:],
                                     start=True, stop=False)
                    nc.tensor.matmul(pr[:], C[f"Wrbf2{l}"][:], ret[:], start=False, stop=True)
                    xkr = sb.tile([128, 512], DT, tag="xkr")
                    # xkr = relu(pr) * kj
                    nc.vector.scalar_tensor_tensor(xkr[:], pr[:], 0.0, kj[:], MAX, MULT)

                    for wi in range(4):
                        w = 4 * sc + wi
                        ws = slice(wi * 128, wi * 128 + 128)
                        # y = relu(xkr_w @ Wdown + b)   (row-major [e,f])
                        py = pss.tile([128, 128], DT, tag="small")
                        nc.tensor.matmul(py[:], C["ones512"][:, :128], C[f"bdownr{l}"][:],
                                         start=True, stop=False)
                        nc.tensor.matmul(py[:], xkr[:, ws], C[f"Wdown{l}"][:],
                                         start=False, stop=True)
                        y_rm = sb.tile([128, 128], DT, tag="y_rm")
                        nc.scalar.activation(y_rm[:], py[:], RELU)

                        trow = sbs.tile([1, 4 * TPW], DT, tag="trow")
                        nc.sync.dma_start(trow[:], d_tripcat[:, w * 4 * TPW:(w + 1) * 4 * TPW])
                        ang_r = trow[:, 0:TPW]
                        dkj_r = trow[:, TPW:2 * TPW]
                        env_r = trow[:, 2 * TPW:3 * TPW]
                        seg_r = trow[:, 3 * TPW:4 * TPW]
                        segc = sbs.tile([128, K_FIX], DT, tag="segc")
                        nc.sync.dma_start(segc[:], d_segcolT[:, w * K_FIX:(w + 1) * K_FIX])

                        # sbf for this window: [NS*NR, TPW]
                        pa = pss.tile([NS, TPW], DT, tag="small")
                        nc.tensor.matmul(pa[:], C["q025"][:], C["ones512"][:, :TPW],
                                         start=True, stop=False)
                        nc.tensor.matmul(pa[:], C["svecn"][:], ang_r,
                                         start=False, stop=True)
                        cbf6 = sin_of_psum(pa, NS, TPW, "c")
                        pb = pss.tile([NR, TPW], DT, tag="small")
                        nc.tensor.matmul(pb[:], C["freqn"][:], dkj_r,
                                         start=True, stop=True)
                        sin16 = sin_of_psum(pb, NR, TPW, "t")
                        pe = pss.tile([NR, TPW], DT, tag="small")
                        nc.tensor.matmul(pe[:], C["ones512"][:, :NR], env_r,
                                         start=True, stop=True)
                        rbf16 = sbs.tile([NR, TPW], DT, tag="rbf16")
                        nc.vector.tensor_tensor(rbf16[:], sin16[:], pe[:], op=MULT)
                        pr96 = psb.tile([NS * NR, TPW], DT, tag="big")
                        nc.tensor.matmul(pr96[:], C["REP6"][:], rbf16[:], start=True, stop=True)
                        pc96 = psb.tile([NS * NR, TPW], DT, tag="big")
                        nc.tensor.matmul(pc96[:], C["REPC"][:], cbf6[:], start=True, stop=True)
                        cbf96 = sb.tile([NS * NR, TPW], DT, tag="cbf96")
                        nc.scalar.copy(cbf96[:], pc96[:])
                        sbf = sb.tile([NS * NR, TPW], DT, tag="sbf")
                        nc.vector.tensor_tensor(sbf[:], pr96[:], cbf96[:], op=MULT)

                        ps1 = psb.tile([128, TPW], DT, tag="big")
                        nc.tensor.matmul(ps1[:], C[f"Wsbf1{l}"][:], sbf[:], start=True, stop=True)
                        s1 = sb.tile([128, TPW], DT, tag="s1")
                        nc.scalar.activation(s1[:], ps1[:], RELU)
                        ps2 = psb.tile([128, TPW], DT, tag="big")
                        nc.tensor.matmul(ps2[:], C[f"Wsbf2{l}"][:], s1[:], start=True, stop=True)
                        s2 = sb.tile([128, TPW], DT, tag="s2")
                        nc.scalar.activation(s2[:], ps2[:], RELU)

                        pagg = psagg.tile([128, 128], DT, tag="agg")
                        for k in range(K_FIX):
                            ks = slice(k * 128, k * 128 + 128)
                            # E_sub[e,t] = (segrel[t] == e)
                            segb = sbs.tile([128, 128], DT, tag="segb")
                            nc.gpsimd.partition_broadcast(segb[:], seg_r[:, ks])
                            esub = sbs.tile([128, 128], DT, tag="esub")
                            nc.vector.tensor_scalar(esub[:], segb[:],
                                                    C["iota_col"][:, :1], None, ISEQ)
                            # S_sub[t,e] = (segrel[t] == e)
                            ssub = sbs.tile([128, 128], DT, tag="ssub")
                            nc.vector.tensor_scalar(ssub[:], C["iota_mat"][:],
                                                    segc[:, k:k + 1], None, ISEQ)
                            px = pss.tile([128, 128], DT, tag="small")
                            nc.tensor.matmul(px[:], y_rm[:], esub[:], start=True, stop=True)
                            xs = sbs.tile([128, 128], DT, tag="xs")
                            nc.vector.tensor_tensor(xs[:], px[:], s2[:, ks], op=MULT)
                            pz = pss.tile([128, 128], DT, tag="small")
                            nc.tensor.matmul(pz[:], C["ones512"][:, :128], C[f"bupr{l}"][:],
                                             start=True, stop=False)
                            nc.tensor.matmul(pz[:], xs[:], C[f"Wup{l}"][:],
                                             start=False, stop=True)
                            z_rm = sbs.tile([128, 128], DT, tag="z_rm")
                            nc.scalar.activation(z_rm[:], pz[:], RELU)
                            nc.tensor.matmul(pagg[:], z_rm[:], ssub[:],
                                             start=(k == 0), stop=(k == K_FIX - 1))

                        agg = sb.tile([128, 128], DT, tag="agg")
                        nc.vector.tensor_copy(agg[:], pagg[:])
                        p1 = pss.tile([128, 128], DT, tag="small")
                        nc.tensor.matmul(p1[:], C[f"Wres1_{l}"][:], agg[:], start=True, stop=True)
                        r1 = sbs.tile([128, 128], DT, tag="r1")
                        nc.scalar.activation(r1[:], p1[:], RELU, bias=C[f"b_res1_{l}"][:])
                        p2 = pss.tile([128, 128], DT, tag="small")
                        nc.tensor.matmul(p2[:], C[f"Wres2_{l}"][:], r1[:], start=True, stop=True)
                        r2 = sbs.tile([128, 128], DT, tag="r2")
                        nc.scalar.activation(r2[:], p2[:], RELU, bias=C[f"b_res2_{l}"][:])
                        mnew = sb.tile([128, 128], DT, tag="mnew")
                        nc.vector.tensor_tensor(mnew[:], agg[:], r2[:], op=ADD)
                        nc.vector.tensor_tensor(mnew[:], mnew[:], mt[:, ws], op=ADD)
                        nc.sync.dma_start(dst[:, w * 128:(w + 1) * 128], mnew[:])
                        if l == L - 1:
                            pt = pss.tile([128, 128], DT, tag="small")
                            nc.tensor.transpose(pt[:], mnew[:], C["identity"][:])
                            mrm = sbs.tile([128, 128], DT, tag="mrm")
                            nc.vector.tensor_copy(mrm[:], pt[:])
                            nc.sync.dma_start(msgRM[w * 128:(w + 1) * 128, :], mrm[:])

            # ---------------- phase 2: atom aggregation ----------------
            for w in range([NAW, 0][os.environ.get("SKIP_P2") == "1"]):
                pap = psagg.tile([128, 128], DT, tag="agg")
                srt2 = sbs.tile([128, K_A], mybir.dt.int32, tag="srt")
                nc.sync.dma_start(srt2[:], d_srcrow[:, w * K_A:(w + 1) * K_A])
                tgt2 = sbs.tile([128, K_A], DT, tag="tgt")
                nc.sync.dma_start(tgt2[:], d_tgtrel[:, w * K_A:(w + 1) * K_A])
                for k in range(K_A):
                    gath = sbs.tile([128, 128], DT, tag="gath")
                    nc.gpsimd.indirect_dma_start(
                        out=gath[:], out_offset=None,
                        in_=msgRM[:],
                        in_offset=IndirectOffsetOnAxis(ap=srt2[:, k:k + 1], axis=0))
                    sat = sbs.tile([128, 128], DT, tag="sat")
                    nc.vector.tensor_scalar(sat[:], C["iota_mat"][:], tgt2[:, k:k + 1], None, ISEQ)
                    nc.tensor.matmul(pap[:], gath[:], sat[:],
                                     start=(k == 0), stop=(k == K_A - 1))
                apt = sbs.tile([128, 128], DT, tag="apt")
                nc.vector.tensor_copy(apt[:], pap[:])
                blk = w // (NAW // NCORES)
                col = (w % (NAW // NCORES)) * 128
                nc.sync.dma_start(apart[blk, :, col:col + 128], apt[:])

            if os.environ.get("SKIP_COLL") != "1" and os.environ.get("SKIP_P2") != "1":
                nc.gpsimd.collective_compute(
                    "ReduceScatter", ADD,
                    replica_groups=[list(range(NCORES))],
                    ins=[apart.opt()], outs=[asum.opt()])

            # ---------------- phase 3: output ----------------
            for j in range(ASH // 512):
                cs = slice(j * 512, j * 512 + 512)
                afh = sb.tile([128, 512], DT, tag="afh")
                nc.sync.dma_start(afh[:], d_afT[0:128, cs])
                afl = sbs.tile([FA - 128, 512], DT, tag="afl")
                nc.sync.dma_start(afl[:], d_afT[128:FA, cs])
                ams = sb.tile([128, 512], DT, tag="ams")
                nc.sync.dma_start(ams[:], asum[:, cs])
                po = psb.tile([128, 512], DT, tag="big")
                nc.tensor.matmul(po[:], C["Woa1"][:], afh[:], start=True, stop=False)
                nc.tensor.matmul(po[:], C["Woa2"][:], afl[:], start=False, stop=False)
                nc.tensor.matmul(po[:], C["Wom"][:], ams[:], start=False, stop=True)
                ot = sb.tile([128, 512], DT, tag="ot")
                nc.scalar.activation(ot[:], po[:], RELU, bias=C["b_o"][:])
                nc.sync.dma_start(d_out[:, cs], ot[:])

    nc.compile()

    in_maps = []
    for c in range(NCORES):
        p = per_core[c]
        m = {k: v for k, v in const.items()}
        m.update(ibT=p["ibT"], embiT=p["embiT"], embjT=p["embjT"], dE=p["dE"],
                 envE=p["envE"], tripcat=p["tripcat"], segcolT=p["segcolT"],
                 srcrow=p["srcrow"], tgtrel=p["tgtrel"], afT=p["afT"])
        in_maps.append(m)

    res = run_bass_kernel_spmd(nc, in_maps, core_ids=list(range(NCORES)))
    global LAST_RESULTS
    LAST_RESULTS = res

    out = np.zeros((N, H), F32)
    for c in range(NCORES):
        lo = c * ASH
        hi = min(N, lo + ASH)
        if hi > lo:
            out[lo:hi] = res.results[c]["outT"][:, :hi - lo].T
    return out
